# revision 3
# baseline (speedup 1.0000x reference)
"""Grouped-experts SwiGLU MLP (DeepseekV3 style) for Trainium2, 8 NeuronCores.

Sharding: expert-parallel. Core e owns expert e's weights and its static
4096-token split. No collectives — token routing is the host-side slice,
outputs concatenate back in token order.

Data movement: all operands are pre-cast to bf16 and pre-tiled into SBUF
layout on the HOST ([128, chunk, free] with the contraction chunk as the
middle dim), so each weight tensor loads with a single max-width DMA
(contiguous 45KB per partition row) and each 512-token x group loads with
one DMA. No on-device casts (SWDGE cast DMAs measured ~6x slower than
straight descriptor DMAs and dominate kernel startup).

Per-core compute (bf16 matmuls, fp32 PSUM):
  gT[h, t] = wg[d, h].T @ xT[d, t]    16-deep accumulation chains, the pg/pu
  uT[h, t] = wu[d, h].T @ xT[d, t]    chains interleave on the PE queue
  hT[h, t] = silu(gT) * uT            (ACT sigmoid + 2 DVE muls, bf16 store)
  outT[d, t] = wd[h, d-blk].T @ hT[h, t]
      down-proj emitted as PAIRS of interleaved 11-deep accumulation chains
      (two 128-col d-blocks in flight) — measured materially faster than the
      stationary-reuse ordering (one ht chunk feeding 4 moving blocks) and
      than single chains; produces the output transposed, un-transposed on
      the host.

Weight loads: the HW pays ~420ns whenever LDWEIGHTS actually changes address
but streams 512-wide MMs at ~176-213ns when it can pipeline; rotating the
stationary every MM inside interleaved accumulation chains measures fastest
(~266ns/MM amortized), so every matmul below rotates its stationary.
"""

import numpy as np

NUM_EXPERTS = 8
DIM = 2048
HIDDEN = 1408
T_E = 4096  # tokens per expert (static equal splits)

P = 128
TN = 512              # token group width (matmul moving dim; ISA max)
NG = T_E // TN        # 8 token groups
DC = DIM // P         # 16 contraction chunks for the up/gate matmuls
HC = HIDDEN // P      # 11 contraction chunks for the down matmul

_nc_cache = []


def _build_program():
    import concourse.mybir as mybir
    import concourse.tile as tile
    from concourse import bacc

    fp32 = mybir.dt.float32
    bf16 = mybir.dt.bfloat16
    AF = mybir.ActivationFunctionType

    nc = bacc.Bacc("TRN2", target_bir_lowering=False, debug=False)

    xT = nc.dram_tensor("xt", [P, DC, T_E], bf16, kind="ExternalInput")
    wg = nc.dram_tensor("wg", [P, DC, HIDDEN], bf16, kind="ExternalInput")
    wu = nc.dram_tensor("wu", [P, DC, HIDDEN], bf16, kind="ExternalInput")
    wd = nc.dram_tensor("wd", [P, HC, DIM], bf16, kind="ExternalInput")
    out = nc.dram_tensor("out", [DIM, T_E], fp32, kind="ExternalOutput")

    with tile.TileContext(nc) as tc:
        with (
            tc.tile_pool(name="wpool", bufs=1) as wpool,
            tc.tile_pool(name="xpool", bufs=2) as xpool,
            tc.tile_pool(name="hpool", bufs=1) as hpool,
            tc.tile_pool(name="spool", bufs=1) as spool,
            tc.tile_pool(name="opool", bufs=1) as opool,
            tc.tile_pool(name="psum", bufs=2, space="PSUM") as psum_pool,
        ):
            wg_sb = wpool.tile([P, DC, HIDDEN], bf16, tag="wg")
            wu_sb = wpool.tile([P, DC, HIDDEN], bf16, tag="wu")
            wd_sb = wpool.tile([P, HC, DIM], bf16, tag="wd")
            xt0_sb = xpool.tile([P, DC, TN], bf16, tag="xt")
            # Load order matters: the first matmul chain needs wg + xt(g0);
            # wu is read ~3us later, wd not until the first down-projection.
            nc.gpsimd.dma_start(out=wg_sb[:, :, :], in_=wg[:, :, :])
            nc.gpsimd.dma_start(out=xt0_sb[:, :, :], in_=xT[:, :, 0:TN])
            nc.gpsimd.dma_start(out=wu_sb[:, :, :], in_=wu[:, :, :])
            nc.gpsimd.dma_start(out=wd_sb[:, :, :], in_=wd[:, :, :])

            for g in range(NG):
                if g == 0:
                    xt_sb = xt0_sb
                else:
                    xt_sb = xpool.tile([P, DC, TN], bf16, tag="xt")
                    nc.gpsimd.dma_start(
                        out=xt_sb[:, :, :], in_=xT[:, :, g * TN:(g + 1) * TN]
                    )

                ht_sb = hpool.tile([P, HC, TN], bf16, tag="ht")
                for hh in range(HC):
                    pg = psum_pool.tile([P, TN], fp32, tag="pg")
                    pu = psum_pool.tile([P, TN], fp32, tag="pu")
                    for c in range(DC):
                        nc.tensor.matmul(
                            pg,
                            wg_sb[:, c, hh * P:(hh + 1) * P],
                            xt_sb[:, c, :],
                            start=(c == 0),
                            stop=(c == DC - 1),
                        )
                    for c in range(DC):
                        nc.tensor.matmul(
                            pu,
                            wu_sb[:, c, hh * P:(hh + 1) * P],
                            xt_sb[:, c, :],
                            start=(c == 0),
                            stop=(c == DC - 1),
                        )
                    # silu(g)*u = (g * sigmoid(g)) * u. Each DVE op reads at
                    # most one PSUM operand (HW limit NCC_IBVF027); Silu LUT
                    # isn't in CoreSim so sigmoid+mul keeps this sim-testable.
                    sig = spool.tile([P, TN], fp32, tag="sig")
                    sil = spool.tile([P, TN], fp32, tag="sil")
                    nc.scalar.activation(sig, pg, AF.Sigmoid)
                    nc.vector.tensor_mul(sil, pg, sig)
                    nc.vector.tensor_mul(ht_sb[:, hh, :], sil, pu)

                # Down-projection: pairs of interleaved 11-deep chains.
                # PSUM: pg(2) + pu(2) + poA(2) + poB(2) = 8 banks.
                for dp in range(DIM // P // 2):
                    poA = psum_pool.tile([P, TN], fp32, tag="poA")
                    poB = psum_pool.tile([P, TN], fp32, tag="poB")
                    for hh in range(HC):
                        nc.tensor.matmul(
                            poA,
                            wd_sb[:, hh, (2 * dp) * P:(2 * dp + 1) * P],
                            ht_sb[:, hh, :],
                            start=(hh == 0),
                            stop=(hh == HC - 1),
                        )
                        nc.tensor.matmul(
                            poB,
                            wd_sb[:, hh, (2 * dp + 1) * P:(2 * dp + 2) * P],
                            ht_sb[:, hh, :],
                            start=(hh == 0),
                            stop=(hh == HC - 1),
                        )
                    otA = opool.tile([P, TN], fp32, tag="otA", bufs=2)
                    otB = opool.tile([P, TN], fp32, tag="otB", bufs=2)
                    nc.vector.tensor_copy(otA, poA)
                    nc.vector.tensor_copy(otB, poB)
                    nc.sync.dma_start(
                        out=out[(2 * dp) * P:(2 * dp + 1) * P, g * TN:(g + 1) * TN],
                        in_=otA,
                    )
                    nc.sync.dma_start(
                        out=out[(2 * dp + 1) * P:(2 * dp + 2) * P, g * TN:(g + 1) * TN],
                        in_=otB,
                    )

    nc.compile()
    return nc


def _get_program():
    if not _nc_cache:
        _nc_cache.append(_build_program())
    return _nc_cache[0]


def _bf16():
    import concourse.mybir as mybir

    return mybir.dt.np(mybir.dt.bfloat16)


def _tile_contract(w, nchunks):
    """[nchunks*128, F] fp32 -> [128, nchunks, F] bf16 (SBUF layout)."""
    bf16 = _bf16()
    F = w.shape[1]
    return np.ascontiguousarray(
        w.reshape(nchunks, P, F).transpose(1, 0, 2).astype(bf16)
    )


def _make_in_maps(inputs):
    x = np.asarray(inputs["x"], dtype=np.float32)
    w_gate = np.asarray(inputs["w_gate"], dtype=np.float32)
    w_up = np.asarray(inputs["w_up"], dtype=np.float32)
    w_down = np.asarray(inputs["w_down"], dtype=np.float32)
    bf16 = _bf16()
    xe = x.reshape(NUM_EXPERTS, T_E, DIM)
    in_maps = []
    for e in range(NUM_EXPERTS):
        # xtt[dd, c, t] = x[t, c*128+dd]
        xtt = np.ascontiguousarray(
            xe[e].reshape(T_E, DC, P).transpose(2, 1, 0).astype(bf16)
        )
        in_maps.append(
            {
                "xt": xtt,
                "wg": _tile_contract(w_gate[e], DC),
                "wu": _tile_contract(w_up[e], DC),
                "wd": _tile_contract(w_down[e], HC),
            }
        )
    return in_maps


def kernel(x, num_tokens_per_expert, w_gate, w_up, w_down, **_ignored):
    from concourse.bass_utils import run_bass_kernel_spmd

    nc = _get_program()
    in_maps = _make_in_maps(
        {"x": x, "w_gate": w_gate, "w_up": w_up, "w_down": w_down}
    )

    res = run_bass_kernel_spmd(nc, in_maps, core_ids=list(range(NUM_EXPERTS)))
    outs = [
        np.ascontiguousarray(np.asarray(r["out"], dtype=np.float32).T)
        for r in res.results
    ]
    return np.concatenate(outs, axis=0)


# revision 4
# speedup vs baseline: 1.0333x; 1.0333x over previous
"""Grouped-experts SwiGLU MLP (DeepseekV3 style) for Trainium2, 8 NeuronCores.

Sharding: expert-parallel. Core e owns expert e's weights and its static
4096-token split. No collectives — token routing is the host-side slice,
outputs concatenate back in token order.

Data movement: all operands are pre-cast to bf16 and pre-tiled into SBUF
layout on the HOST ([128, chunk, free] with the contraction chunk as the
middle dim), so each weight tensor loads with a single max-width DMA
(contiguous 45KB per partition row) and each 512-token x group loads with
one DMA. No on-device casts (SWDGE cast DMAs measured ~6x slower than
straight descriptor DMAs and dominate kernel startup).

Per-core compute (bf16 matmuls, fp32 PSUM):
  gT[h, t] = wg[d, h].T @ xT[d, t]    16-deep accumulation chains, the pg/pu
  uT[h, t] = wu[d, h].T @ xT[d, t]    chains interleave on the PE queue
  hT[h, t] = silu(gT) * uT            (ACT sigmoid + 2 DVE muls, bf16 store)
  outT[d, t] = wd[h, d-blk].T @ hT[h, t]
      down-proj emitted as PAIRS of interleaved 11-deep accumulation chains
      (two 128-col d-blocks in flight) — measured materially faster than the
      stationary-reuse ordering (one ht chunk feeding 4 moving blocks) and
      than single chains; produces the output transposed, un-transposed on
      the host.

Weight loads: the HW pays ~420ns whenever LDWEIGHTS actually changes address
but streams 512-wide MMs at ~176-213ns when it can pipeline; rotating the
stationary every MM inside interleaved accumulation chains measures fastest
(~266ns/MM amortized), so every matmul below rotates its stationary.
"""

import numpy as np

NUM_EXPERTS = 8
DIM = 2048
HIDDEN = 1408
T_E = 4096  # tokens per expert (static equal splits)

P = 128
TN = 512              # token group width (matmul moving dim; ISA max)
NG = T_E // TN        # 8 token groups
DC = DIM // P         # 16 contraction chunks for the up/gate matmuls
HC = HIDDEN // P      # 11 contraction chunks for the down matmul

_nc_cache = []


def _build_program():
    import concourse.mybir as mybir
    import concourse.tile as tile
    from concourse import bacc

    fp32 = mybir.dt.float32
    bf16 = mybir.dt.bfloat16
    AF = mybir.ActivationFunctionType

    nc = bacc.Bacc("TRN2", target_bir_lowering=False, debug=False)

    xT = nc.dram_tensor("xt", [P, DC, T_E], bf16, kind="ExternalInput")
    wg = nc.dram_tensor("wg", [P, DC, HIDDEN], bf16, kind="ExternalInput")
    wu = nc.dram_tensor("wu", [P, DC, HIDDEN], bf16, kind="ExternalInput")
    wd = nc.dram_tensor("wd", [P, HC, DIM], bf16, kind="ExternalInput")
    out = nc.dram_tensor("out", [DIM, T_E], fp32, kind="ExternalOutput")

    with tile.TileContext(nc) as tc:
        with (
            tc.tile_pool(name="wpool", bufs=1) as wpool,
            tc.tile_pool(name="xpool", bufs=2) as xpool,
            tc.tile_pool(name="hpool", bufs=1) as hpool,
            tc.tile_pool(name="spool", bufs=1) as spool,
            tc.tile_pool(name="opool", bufs=1) as opool,
            tc.tile_pool(name="psum", bufs=2, space="PSUM") as psum_pool,
        ):
            wg_sb = wpool.tile([P, DC, HIDDEN], bf16, tag="wg")
            wu_sb = wpool.tile([P, DC, HIDDEN], bf16, tag="wu")
            wd_sb = wpool.tile([P, HC, DIM], bf16, tag="wd")
            xt0_sb = xpool.tile([P, DC, TN], bf16, tag="xt")
            # Load order matters: the first matmul chain needs wg + xt(g0);
            # wu is read ~3us later, wd not until the first down-projection.
            nc.gpsimd.dma_start(out=wg_sb[:, :, :], in_=wg[:, :, :])
            nc.gpsimd.dma_start(out=xt0_sb[:, :, :], in_=xT[:, :, 0:TN])
            nc.gpsimd.dma_start(out=wu_sb[:, :, :], in_=wu[:, :, :])
            nc.gpsimd.dma_start(out=wd_sb[:, :, :], in_=wd[:, :, :])

            for g in range(NG):
                if g == 0:
                    xt_sb = xt0_sb
                else:
                    xt_sb = xpool.tile([P, DC, TN], bf16, tag="xt")
                    nc.gpsimd.dma_start(
                        out=xt_sb[:, :, :], in_=xT[:, :, g * TN:(g + 1) * TN]
                    )

                ht_sb = hpool.tile([P, HC, TN], bf16, tag="ht")
                for hh in range(HC):
                    pg = psum_pool.tile([P, TN], fp32, tag="pg")
                    pu = psum_pool.tile([P, TN], fp32, tag="pu")
                    for c in range(DC):
                        nc.tensor.matmul(
                            pg,
                            wg_sb[:, c, hh * P:(hh + 1) * P],
                            xt_sb[:, c, :],
                            start=(c == 0),
                            stop=(c == DC - 1),
                        )
                    for c in range(DC):
                        nc.tensor.matmul(
                            pu,
                            wu_sb[:, c, hh * P:(hh + 1) * P],
                            xt_sb[:, c, :],
                            start=(c == 0),
                            stop=(c == DC - 1),
                        )
                    # silu(g)*u = (g * sigmoid(g)) * u. Each DVE op reads at
                    # most one PSUM operand (HW limit NCC_IBVF027); Silu LUT
                    # isn't in CoreSim so sigmoid+mul keeps this sim-testable.
                    sig = spool.tile([P, TN], fp32, tag="sig")
                    sil = spool.tile([P, TN], fp32, tag="sil")
                    nc.scalar.activation(sig, pg, AF.Sigmoid)
                    nc.vector.tensor_mul(sil, pg, sig)
                    nc.vector.tensor_mul(ht_sb[:, hh, :], sil, pu)

                # Down-projection: FOUR interleaved 11-deep accumulation
                # chains (measured fastest; 2-way was +75us, 1-way +500us).
                # PSUM: pg(2) + pu(2) + po0..3(1 each) = 8 banks.
                for dq in range(DIM // P // 4):
                    po0 = psum_pool.tile([P, TN], fp32, tag="po0", bufs=1)
                    po1 = psum_pool.tile([P, TN], fp32, tag="po1", bufs=1)
                    po2 = psum_pool.tile([P, TN], fp32, tag="po2", bufs=1)
                    po3 = psum_pool.tile([P, TN], fp32, tag="po3", bufs=1)
                    pos = [po0, po1, po2, po3]
                    for hh in range(HC):
                        for i in range(4):
                            nc.tensor.matmul(
                                pos[i],
                                wd_sb[:, hh, (4 * dq + i) * P:(4 * dq + i + 1) * P],
                                ht_sb[:, hh, :],
                                start=(hh == 0),
                                stop=(hh == HC - 1),
                            )
                    for i in range(4):
                        oti = opool.tile([P, TN], fp32, tag="ot" + str(i), bufs=2)
                        nc.vector.tensor_copy(oti, pos[i])
                        nc.sync.dma_start(
                            out=out[(4 * dq + i) * P:(4 * dq + i + 1) * P, g * TN:(g + 1) * TN],
                            in_=oti,
                        )

    nc.compile()
    return nc


def _get_program():
    if not _nc_cache:
        _nc_cache.append(_build_program())
    return _nc_cache[0]


def _bf16():
    import concourse.mybir as mybir

    return mybir.dt.np(mybir.dt.bfloat16)


def _tile_contract(w, nchunks):
    """[nchunks*128, F] fp32 -> [128, nchunks, F] bf16 (SBUF layout)."""
    bf16 = _bf16()
    F = w.shape[1]
    return np.ascontiguousarray(
        w.reshape(nchunks, P, F).transpose(1, 0, 2).astype(bf16)
    )


def _make_in_maps(inputs):
    x = np.asarray(inputs["x"], dtype=np.float32)
    w_gate = np.asarray(inputs["w_gate"], dtype=np.float32)
    w_up = np.asarray(inputs["w_up"], dtype=np.float32)
    w_down = np.asarray(inputs["w_down"], dtype=np.float32)
    bf16 = _bf16()
    xe = x.reshape(NUM_EXPERTS, T_E, DIM)
    in_maps = []
    for e in range(NUM_EXPERTS):
        # xtt[dd, c, t] = x[t, c*128+dd]
        xtt = np.ascontiguousarray(
            xe[e].reshape(T_E, DC, P).transpose(2, 1, 0).astype(bf16)
        )
        in_maps.append(
            {
                "xt": xtt,
                "wg": _tile_contract(w_gate[e], DC),
                "wu": _tile_contract(w_up[e], DC),
                "wd": _tile_contract(w_down[e], HC),
            }
        )
    return in_maps


def kernel(x, num_tokens_per_expert, w_gate, w_up, w_down, **_ignored):
    from concourse.bass_utils import run_bass_kernel_spmd

    nc = _get_program()
    in_maps = _make_in_maps(
        {"x": x, "w_gate": w_gate, "w_up": w_up, "w_down": w_down}
    )

    res = run_bass_kernel_spmd(nc, in_maps, core_ids=list(range(NUM_EXPERTS)))
    outs = [
        np.ascontiguousarray(np.asarray(r["out"], dtype=np.float32).T)
        for r in res.results
    ]
    return np.concatenate(outs, axis=0)
